# revision 9
# baseline (speedup 1.0000x reference)
"""Trainium2 Bass kernel for one FDM wave-equation step (5-point stencil CNN).

u2 = 2*u1 - u0 + 0.25*lap5(u1) - 0.0025*(j2 - j0)   on (16,1,1024,1024) f32.

Sharding: data-parallel over batch - 2 full images per NeuronCore. The result
tolerance (2e-2 L2) admits low-precision I/O, which is the main lever since the
problem is HBM-bandwidth bound:

  u1  -> bf16, pre-scaled by 0.25 (exact power-of-2) and zero-padded by one
         column on each side (so the horizontal stencil is a pure add with
         free edge handling)
  u0, j2, j0 -> fp8 e3m4, packed side by side into one [rows, 3*W] dram
         tensor (one DMA + one HWDGE descriptor-gen per tile instead of 3;
         HWDGE is a single serialized device in the cost model)
  out -> bf16

Per 126-row tile: all the linear terms except the horizontal neighbors run on
the TensorEngine into one PSUM group: the vertical stencil + center term as a
banded-matrix matmul over the tile's u1 rows (the missing top-neighbor row is
stashed at partition 127 by a tiny gpsimd DMA and fed to output row 0 by a
band-matrix entry at [127, 0]), u0 via a -I matmul, j2/j0 via -+c*I diagonal
matmuls on the fp8 data. The ACT engine drains PSUM to a bf16 tile, the DVE
adds the horizontal (left+right) neighbor sum with two tensor_tensor adds
(both at DVE 2x rate), and the store goes out on the Pool SWDGE ring to stay
off HWDGE.

Measured end-to-end rel err vs the fp32 reference: ~9.2e-3.
"""

import numpy as np
import ml_dtypes

import concourse.bacc as bacc
import concourse.mybir as mybir
import concourse.tile as tile
from concourse import bass_utils

F32 = mybir.dt.float32
BF16 = mybir.dt.bfloat16
F8E3 = mybir.dt.float8e3
ALU = mybir.AluOpType
NP_BF16 = ml_dtypes.bfloat16
NP_F8E3 = ml_dtypes.float8_e3m4

H = W = 1024
B = 16
NCORES = 8
IMGS_PER_CORE = B // NCORES          # 2
ROWS = IMGS_PER_CORE * H             # 2048 rows per core
WP = W + 2                           # u1 padded width
TS = 126                             # output rows per tile
NTILES = (H + TS - 1) // TS          # 9
C_J = 0.0025                         # DT / (2*EPSILON)

# u1 is shipped pre-scaled by C_LAP=0.25, so the stencil weights on the
# scaled field are: center (2-4*0.25)/0.25 = 4, neighbors 1.
W_CENTER = 4.0
W_NEIGH = 1.0


def _const_matrices():
    # bandT[k, m]: weight of u1 partition k (image row base+k) on output row
    # m. Top-edge zero-pad: row 0 simply has no k=-1 entry. Bottom-edge
    # zero-pad falls out of slicing the contraction down to the rows present.
    bandT = np.zeros((128, 128), dtype=NP_BF16)
    for m in range(128):
        if m >= 1:
            bandT[m - 1, m] = W_NEIGH
        bandT[m, m] = W_CENTER
        if m + 1 < 128:
            bandT[m + 1, m] = W_NEIGH
    # bandTH: same, plus the top-neighbor row stashed at partition 127
    # feeding output row 0 (used for every tile but the first).
    bandTH = bandT.copy()
    bandTH[127, 0] = W_NEIGH
    dj2 = (-C_J * np.eye(128)).astype(NP_BF16)
    dj0 = (C_J * np.eye(128)).astype(NP_BF16)
    # bandT | bandTH | dj2 | dj0 packed column-wise: one DMA for all four
    cbf = np.concatenate([bandT, bandTH, dj2, dj0], axis=1)
    negi = (-np.eye(128)).astype(NP_F8E3)
    return cbf, negi


def _build_program():
    nc = bacc.Bacc(
        "TRN2",
        debug=False,
        enable_asserts=False,
        target_bir_lowering=False,
        num_devices=NCORES,
    )
    u1d = nc.dram_tensor("u1", [ROWS, WP], BF16, kind="ExternalInput").ap()
    # u0 | j2 | j0 packed along the row
    pkd = nc.dram_tensor("pk", [ROWS, 3 * W], F8E3, kind="ExternalInput").ap()
    outd = nc.dram_tensor("out", [ROWS, W], BF16, kind="ExternalOutput").ap()

    cbf_np, negi_np = _const_matrices()
    cbf_d = nc.inline_tensor(cbf_np, name="cbf")
    negi_d = nc.inline_tensor(negi_np, name="negi")

    with tile.TileContext(nc) as tc:
        with tc.tile_pool(name="consts", bufs=1) as cpool, \
             tc.tile_pool(name="io", bufs=6) as iopool, \
             tc.tile_pool(name="res", bufs=6) as rpool, \
             tc.tile_pool(name="ps", bufs=4, space="PSUM") as pspool:
            cbf = cpool.tile([128, 512], BF16, name="cbf_sb")
            negi = cpool.tile([128, 128], F8E3, name="negi_sb")
            bandT = cbf[:, 0:128]
            bandTH = cbf[:, 128:256]
            dj2 = cbf[:, 256:384]
            dj0 = cbf[:, 384:512]
            consts_loaded = False

            for img in range(IMGS_PER_CORE):
                r0 = H * img
                for t in range(NTILES):
                    base = TS * t
                    M = min(TS, H - base)
                    K1 = min(M + 1, H - base)   # rows incl. bottom neighbor

                    u1t = iopool.tile([128, WP], BF16, name="u1t")
                    nc.sync.dma_start(u1t[0:K1], u1d[r0 + base:r0 + base + K1, :])
                    pkt = iopool.tile([128, 3 * W], F8E3, name="pkt")
                    nc.scalar.dma_start(
                        pkt[0:M], pkd[r0 + base:r0 + base + M, :])
                    if t == 0:
                        K, band = K1, bandT
                    else:
                        # top-neighbor u1 row rides at partition 127 (tiny
                        # SWDGE DMA: keep it off the serialized HWDGE device)
                        nc.gpsimd.dma_start(
                            u1t[127:128], u1d[r0 + base - 1:r0 + base, :])
                        K, band = 128, bandTH
                    if not consts_loaded:
                        # const loads ride the SWDGE ring (the serialized
                        # HWDGE device delays tile loads otherwise) after the
                        # first big loads so descriptor-gen feeds data at once
                        nc.gpsimd.dma_start(cbf[:], cbf_d.ap())
                        nc.gpsimd.dma_start(negi[:], negi_d.ap())
                        consts_loaded = True

                    # PSUM accumulates everything linear except the
                    # horizontal neighbors: band@u1' - u0 - cj*j2 + cj*j0.
                    ps = pspool.tile([128, W], F32, name="ps")
                    for h in range(2):
                        cs = slice(512 * h, 512 * h + 512)
                        nc.tensor.matmul(
                            ps[0:M, cs], band[0:K, 0:M],
                            u1t[0:K, 1 + 512 * h:513 + 512 * h],
                            start=True, stop=False,
                        )
                        nc.tensor.matmul(
                            ps[0:M, cs], negi[0:M, 0:M],
                            pkt[0:M, 512 * h:512 * h + 512],
                            start=False, stop=False,
                        )
                        nc.tensor.matmul(
                            ps[0:M, cs], dj2[0:M, 0:M],
                            pkt[0:M, W + 512 * h:W + 512 * h + 512],
                            start=False, stop=False,
                        )
                        nc.tensor.matmul(
                            ps[0:M, cs], dj0[0:M, 0:M],
                            pkt[0:M, 2 * W + 512 * h:2 * W + 512 * h + 512],
                            start=False, stop=True,
                        )

                    # tmp = u1'[., x-1] + u1'[., x+1] (edge zero-pad via the
                    # host-padded columns)
                    tmp = rpool.tile([128, W], BF16, name="tmp")
                    nc.vector.tensor_tensor(
                        tmp[0:M], u1t[0:M, 0:W], u1t[0:M, 2:WP], ALU.add)
                    # rt = psum, then rt += tmp
                    rt = rpool.tile([128, W], BF16, name="rt")
                    nc.scalar.copy(rt[0:M], ps[0:M])
                    nc.vector.tensor_tensor(
                        rt[0:M], rt[0:M], tmp[0:M], ALU.add)

                    nc.gpsimd.dma_start(
                        outd[r0 + base:r0 + base + M, :], rt[0:M])

    nc.compile()
    return nc


_NC_CACHE = None


def _get_program():
    global _NC_CACHE
    if _NC_CACHE is None:
        _NC_CACHE = _build_program()
    return _NC_CACHE


def kernel(u1, u0, j2, j0):
    nc = _get_program()

    u1p = np.zeros((B, H, WP), dtype=NP_BF16)
    u1p[:, :, 1:W + 1] = (0.25 * u1.reshape(B, H, W)).astype(NP_BF16)
    pk = np.empty((B, H, 3 * W), dtype=NP_F8E3)
    pk[:, :, 0:W] = u0.reshape(B, H, W).astype(NP_F8E3)
    pk[:, :, W:2 * W] = j2.reshape(B, H, W).astype(NP_F8E3)
    pk[:, :, 2 * W:] = j0.reshape(B, H, W).astype(NP_F8E3)

    in_maps = []
    for c in range(NCORES):
        sl = slice(IMGS_PER_CORE * c, IMGS_PER_CORE * (c + 1))
        in_maps.append({
            "u1": np.ascontiguousarray(u1p[sl]).reshape(ROWS, WP),
            "pk": np.ascontiguousarray(pk[sl]).reshape(ROWS, 3 * W),
        })
    res = bass_utils.run_bass_kernel_spmd(nc, in_maps, core_ids=list(range(NCORES)))
    out = np.concatenate(
        [r["out"].reshape(IMGS_PER_CORE, 1, H, W) for r in res.results], axis=0
    )
    return out.astype(np.float32)


# revision 10
# speedup vs baseline: 1.0156x; 1.0156x over previous
"""Trainium2 Bass kernel for one FDM wave-equation step (5-point stencil CNN).

u2 = 2*u1 - u0 + 0.25*lap5(u1) - 0.0025*(j2 - j0)   on (16,1,1024,1024) f32.

Sharding: data-parallel over batch - 2 full images per NeuronCore. The result
tolerance (2e-2 L2) admits low-precision I/O, which is the main lever since the
problem is HBM-bandwidth bound (the TimelineSim cost model moves bytes at
360 GB/s through a serialized DMA-engine pool):

  u1  -> bf16, pre-scaled by 0.25 (exact power-of-2) and zero-padded by one
         column on each side, so the horizontal stencil is a pure add with
         free edge handling and the stencil weights are {1, 4}
  u0  -> fp8 e3m4 (4 mantissa bits; e4m3 would cost ~1.8e-2 rel err, e3m4
         only ~9e-3)
  j2/j0 -> fp8 e4m3, interleaved per row as [j2h0|j0h0|j2h1|j0h1] blocks of
         512 so one DoubleRow matmul applies both coefficients per half
  out -> bf16

Per 126-row tile: all linear terms except the horizontal neighbors accumulate
in one PSUM group on the TensorEngine: the vertical stencil + center as a
banded-matrix matmul over the tile's u1 rows (the missing top-neighbor row is
stashed at partition 127 by a tiny Pool-ring DMA and fed to output row 0 by a
band entry at [127, 0]), u0 via a -I matmul (e3m4), and j2/j0 via a single
fp8 DoubleRow matmul with +-cj diagonal weights in e5m2. The ACT engine
drains PSUM to bf16, the DVE adds the horizontal neighbor sum (two 2x-rate
tensor_tensor adds), and stores ride the Pool SWDGE ring to stay off the
serialized HWDGE descriptor-gen device, which the input loads keep busy.

Measured end-to-end rel err vs the fp32 reference: ~9.3e-3.
"""

import numpy as np
import ml_dtypes

import concourse.bacc as bacc
import concourse.mybir as mybir
import concourse.tile as tile
from concourse import bass_utils

F32 = mybir.dt.float32
BF16 = mybir.dt.bfloat16
F8E3 = mybir.dt.float8e3
F8E4 = mybir.dt.float8e4
F8E5 = mybir.dt.float8e5
ALU = mybir.AluOpType
DR = mybir.MatmulPerfMode.DoubleRow
NP_BF16 = ml_dtypes.bfloat16
NP_F8E3 = ml_dtypes.float8_e3m4
NP_F8E4 = ml_dtypes.float8_e4m3
NP_F8E5 = ml_dtypes.float8_e5m2

H = W = 1024
B = 16
NCORES = 8
IMGS_PER_CORE = B // NCORES          # 2
ROWS = IMGS_PER_CORE * H             # 2048 rows per core
WP = W + 2                           # u1 padded width
TS = 126                             # output rows per tile
NTILES = (H + TS - 1) // TS          # 9
C_J = 0.0025                         # DT / (2*EPSILON)

# u1 is shipped pre-scaled by C_LAP=0.25, so the stencil weights on the
# scaled field are: center (2-4*0.25)/0.25 = 4, neighbors 1.
W_CENTER = 4.0
W_NEIGH = 1.0


def _const_matrices():
    # bandT[k, m]: weight of u1 partition k (image row base+k) on output row
    # m. Top-edge zero-pad: row 0 simply has no k=-1 entry. Bottom-edge
    # zero-pad falls out of slicing the contraction down to the rows present.
    bandT = np.zeros((128, 128), dtype=NP_BF16)
    for m in range(128):
        if m >= 1:
            bandT[m - 1, m] = W_NEIGH
        bandT[m, m] = W_CENTER
        if m + 1 < 128:
            bandT[m + 1, m] = W_NEIGH
    # bandTH: same, plus the top-neighbor row stashed at partition 127
    # feeding output row 0 (used for every tile but the first).
    bandTH = bandT.copy()
    bandTH[127, 0] = W_NEIGH
    cbf = np.concatenate([bandT, bandTH], axis=1)
    negi = (-np.eye(128)).astype(NP_F8E3)
    # DoubleRow diag weights: k-tile 0 applies -cj to j2, k-tile 1 +cj to j0.
    # cj in e5m2 is 2.3% off 0.0025; at a 0.0025-weighted term that shifts
    # the output by ~5e-5 relative - irrelevant.
    cjq = np.float32(NP_F8E5(C_J))
    djdr = np.zeros((128, 2, 128), dtype=NP_F8E5)
    for m in range(128):
        djdr[m, 0, m] = NP_F8E5(-cjq)
        djdr[m, 1, m] = NP_F8E5(cjq)
    return cbf, negi, djdr


def _build_program():
    nc = bacc.Bacc(
        "TRN2",
        debug=False,
        enable_asserts=False,
        target_bir_lowering=False,
        num_devices=NCORES,
    )
    u1d = nc.dram_tensor("u1", [ROWS, WP], BF16, kind="ExternalInput").ap()
    u0d = nc.dram_tensor("u0", [ROWS, W], F8E3, kind="ExternalInput").ap()
    # j2 | j0 | j2 | j0 in 512-column blocks (DoubleRow k-tile pairs)
    jd = nc.dram_tensor("jd", [ROWS, 4, 512], F8E4, kind="ExternalInput").ap()
    outd = nc.dram_tensor("out", [ROWS, W], BF16, kind="ExternalOutput").ap()

    cbf_np, negi_np, djdr_np = _const_matrices()
    cbf_d = nc.inline_tensor(cbf_np, name="cbf")
    negi_d = nc.inline_tensor(negi_np, name="negi")
    djdr_d = nc.inline_tensor(djdr_np, name="djdr")

    with tile.TileContext(nc) as tc:
        with tc.tile_pool(name="consts", bufs=1) as cpool, \
             tc.tile_pool(name="pu1", bufs=4) as pu1, \
             tc.tile_pool(name="pu0", bufs=4) as pu0, \
             tc.tile_pool(name="pjt", bufs=4) as pjt, \
             tc.tile_pool(name="ptmp", bufs=4) as ptmp, \
             tc.tile_pool(name="prt", bufs=4) as prt, \
             tc.tile_pool(name="ps", bufs=4, space="PSUM") as pspool:
            cbf = cpool.tile([128, 256], BF16, name="cbf_sb")
            negi = cpool.tile([128, 128], F8E3, name="negi_sb")
            djdr = cpool.tile([128, 2, 128], F8E5, name="djdr_sb")
            bandT = cbf[:, 0:128]
            bandTH = cbf[:, 128:256]
            consts_loaded = False

            for img in range(IMGS_PER_CORE):
                r0 = H * img
                for t in range(NTILES):
                    base = TS * t
                    M = min(TS, H - base)
                    K1 = min(M + 1, H - base)   # rows incl. bottom neighbor

                    u1t = pu1.tile([128, WP], BF16, name="u1t")
                    nc.sync.dma_start(u1t[0:K1], u1d[r0 + base:r0 + base + K1, :])
                    u0t = pu0.tile([128, W], F8E3, name="u0t")
                    nc.sync.dma_start(u0t[0:M], u0d[r0 + base:r0 + base + M, :])
                    jt = pjt.tile([128, 4, 512], F8E4, name="jt")
                    nc.sync.dma_start(jt[0:M], jd[r0 + base:r0 + base + M])
                    if t == 0:
                        K, band = K1, bandT
                    else:
                        # top-neighbor u1 row rides at partition 127 (tiny
                        # SWDGE DMA: keep it off the serialized HWDGE device)
                        nc.gpsimd.dma_start(
                            u1t[127:128], u1d[r0 + base - 1:r0 + base, :])
                        K, band = 128, bandTH
                    if not consts_loaded:
                        # const loads ride the SWDGE ring (the serialized
                        # HWDGE device delays tile loads otherwise) after the
                        # first big loads so descriptor-gen feeds data at once
                        nc.gpsimd.dma_start(cbf[:], cbf_d.ap())
                        nc.gpsimd.dma_start(negi[:], negi_d.ap())
                        nc.gpsimd.dma_start(djdr[:], djdr_d.ap())
                        consts_loaded = True

                    # PSUM accumulates everything linear except the
                    # horizontal neighbors: band@u1' - u0 - cj*j2 + cj*j0.
                    ps = pspool.tile([128, W], F32, name="ps")
                    for h in range(2):
                        cs = slice(512 * h, 512 * h + 512)
                        nc.tensor.matmul(
                            ps[0:M, cs], band[0:K, 0:M],
                            u1t[0:K, 1 + 512 * h:513 + 512 * h],
                            start=True, stop=False,
                        )
                        nc.tensor.matmul(
                            ps[0:M, cs], negi[0:M, 0:M], u0t[0:M, cs],
                            start=False, stop=False,
                        )
                        nc.tensor.matmul(
                            ps[0:M, cs], djdr[0:M, :, 0:M],
                            jt[0:M, 2 * h:2 * h + 2, :],
                            start=False, stop=True, perf_mode=DR,
                        )

                    # tmp = u1'[., x-1] + u1'[., x+1] (edge zero-pad via the
                    # host-padded columns)
                    tmp = ptmp.tile([128, W], BF16, name="tmp")
                    nc.vector.tensor_tensor(
                        tmp[0:M], u1t[0:M, 0:W], u1t[0:M, 2:WP], ALU.add)
                    # rt = psum, then rt += tmp
                    rt = prt.tile([128, W], BF16, name="rt")
                    nc.scalar.copy(rt[0:M], ps[0:M])
                    nc.vector.tensor_tensor(
                        rt[0:M], rt[0:M], tmp[0:M], ALU.add)

                    nc.gpsimd.dma_start(
                        outd[r0 + base:r0 + base + M, :], rt[0:M])

    nc.compile()
    return nc


_NC_CACHE = None


def _get_program():
    global _NC_CACHE
    if _NC_CACHE is None:
        _NC_CACHE = _build_program()
    return _NC_CACHE


def kernel(u1, u0, j2, j0):
    nc = _get_program()

    u1p = np.zeros((B, H, WP), dtype=NP_BF16)
    u1p[:, :, 1:W + 1] = (0.25 * u1.reshape(B, H, W)).astype(NP_BF16)
    u0q = u0.reshape(B, H, W).astype(NP_F8E3)
    j2q = j2.reshape(B, H, W).astype(NP_F8E4)
    j0q = j0.reshape(B, H, W).astype(NP_F8E4)
    jq = np.empty((B, H, 4, 512), dtype=NP_F8E4)
    jq[:, :, 0, :] = j2q[:, :, 0:512]
    jq[:, :, 1, :] = j0q[:, :, 0:512]
    jq[:, :, 2, :] = j2q[:, :, 512:1024]
    jq[:, :, 3, :] = j0q[:, :, 512:1024]

    in_maps = []
    for c in range(NCORES):
        sl = slice(IMGS_PER_CORE * c, IMGS_PER_CORE * (c + 1))
        in_maps.append({
            "u1": np.ascontiguousarray(u1p[sl]).reshape(ROWS, WP),
            "u0": np.ascontiguousarray(u0q[sl]).reshape(ROWS, W),
            "jd": np.ascontiguousarray(jq[sl]).reshape(ROWS, 4, 512),
        })
    res = bass_utils.run_bass_kernel_spmd(nc, in_maps, core_ids=list(range(NCORES)))
    out = np.concatenate(
        [r["out"].reshape(IMGS_PER_CORE, 1, H, W) for r in res.results], axis=0
    )
    return out.astype(np.float32)


# revision 11
# speedup vs baseline: 1.0807x; 1.0640x over previous
"""Trainium2 Bass kernel for one FDM wave-equation step (5-point stencil CNN).

u2 = 2*u1 - u0 + 0.25*lap5(u1) - 0.0025*(j2 - j0)   on (16,1,1024,1024) f32.

Sharding: data-parallel over batch - 2 full images per NeuronCore. The result
tolerance (2e-2 L2) admits low-precision I/O, which is the main lever since
the problem is HBM-bandwidth bound (the TimelineSim cost model moves bytes at
360 GB/s through a serialized DMA-engine pool):

  u1  -> fp8 e3m4 (4 mantissa bits), zero-padded by one column each side
  u0  -> fp8 e3m4
  j2/j0 -> fp8 e4m3, interleaved per row as [j2h0|j0h0|j2h1|j0h1] blocks of
         512 so one DoubleRow matmul applies both coefficients per half
  out -> bf16, holding 4x the result; the host multiplies by 0.25 (exact)

The 4x output scale makes every device-side constant exact in fp8 with no
extra scaling pass: the stencil weights on raw u1 become {1, 4} (e3m4-exact),
u0's weight -4, and the horizontal neighbor sum u1[x-1]+u1[x+1] needs no
scale at all.

Per 126-row tile: all linear terms except the horizontal neighbors accumulate
in one PSUM group on the TensorEngine: the vertical stencil + center as a
banded-matrix matmul over the tile's u1 rows (the missing top-neighbor row is
stashed at partition 127 by a tiny Pool-ring DMA and fed to output row 0 by a
band entry at [127, 0]), u0 via a -4I matmul, and j2/j0 via a single fp8
DoubleRow matmul with -+4cj diagonal weights in e5m2 (2.3% off 0.01, which
shifts the 0.0025-weighted j-term by a negligible 5e-5 of the output). The
ACT engine drains PSUM to bf16, the DVE adds the horizontal neighbor sum
(two tensor_tensor adds), and stores ride the Pool SWDGE ring to stay off
the serialized HWDGE descriptor-gen device, which the input loads keep busy.

Measured end-to-end rel err vs the fp32 reference: ~1.4e-2 (limit 2e-2).
"""

import numpy as np
import ml_dtypes

import concourse.bacc as bacc
import concourse.mybir as mybir
import concourse.tile as tile
from concourse import bass_utils

F32 = mybir.dt.float32
BF16 = mybir.dt.bfloat16
F8E3 = mybir.dt.float8e3
F8E4 = mybir.dt.float8e4
F8E5 = mybir.dt.float8e5
ALU = mybir.AluOpType
DR = mybir.MatmulPerfMode.DoubleRow
NP_BF16 = ml_dtypes.bfloat16
NP_F8E3 = ml_dtypes.float8_e3m4
NP_F8E4 = ml_dtypes.float8_e4m3
NP_F8E5 = ml_dtypes.float8_e5m2

H = W = 1024
B = 16
NCORES = 8
IMGS_PER_CORE = B // NCORES          # 2
ROWS = IMGS_PER_CORE * H             # 2048 rows per core
WP = W + 2                           # u1 padded width
TS = 126                             # output rows per tile
NTILES = (H + TS - 1) // TS          # 9
C_J = 0.0025                         # DT / (2*EPSILON)


def _const_matrices():
    # bandT[k, m]: weight of u1 partition k (image row base+k) on 4x output
    # row m: {1, 4, 1} tridiagonal, all e3m4-exact. Top-edge zero-pad: row 0
    # has no k=-1 entry. Bottom-edge zero-pad falls out of slicing the
    # contraction down to the rows present.
    bandT = np.zeros((128, 128), dtype=NP_F8E3)
    for m in range(128):
        if m >= 1:
            bandT[m - 1, m] = NP_F8E3(1.0)
        bandT[m, m] = NP_F8E3(4.0)
        if m + 1 < 128:
            bandT[m + 1, m] = NP_F8E3(1.0)
    # bandTH: same, plus the top-neighbor row stashed at partition 127
    # feeding output row 0 (used for every tile but the first).
    bandTH = bandT.copy()
    bandTH[127, 0] = NP_F8E3(1.0)
    negi4 = (-4.0 * np.eye(128)).astype(NP_F8E3)
    ce3 = np.concatenate([bandT, bandTH, negi4], axis=1)   # one DMA
    # DoubleRow diag weights: k-tile 0 applies -4cj to j2, k-tile 1 +4cj to
    # j0 (on the 4x-scaled output).
    cj4 = np.float32(NP_F8E5(4 * C_J))
    djdr = np.zeros((128, 2, 128), dtype=NP_F8E5)
    for m in range(128):
        djdr[m, 0, m] = NP_F8E5(-cj4)
        djdr[m, 1, m] = NP_F8E5(cj4)
    return ce3, djdr


def _build_program():
    nc = bacc.Bacc(
        "TRN2",
        debug=False,
        enable_asserts=False,
        target_bir_lowering=False,
        num_devices=NCORES,
    )
    u1d = nc.dram_tensor("u1", [ROWS, WP], F8E3, kind="ExternalInput").ap()
    u0d = nc.dram_tensor("u0", [ROWS, W], F8E3, kind="ExternalInput").ap()
    # j2 | j0 | j2 | j0 in 512-column blocks (DoubleRow k-tile pairs)
    jd = nc.dram_tensor("jd", [ROWS, 4, 512], F8E4, kind="ExternalInput").ap()
    outd = nc.dram_tensor("out", [ROWS, W], BF16, kind="ExternalOutput").ap()

    ce3_np, djdr_np = _const_matrices()
    ce3_d = nc.inline_tensor(ce3_np, name="ce3")
    djdr_d = nc.inline_tensor(djdr_np, name="djdr")

    with tile.TileContext(nc) as tc:
        with tc.tile_pool(name="consts", bufs=1) as cpool, \
             tc.tile_pool(name="pu1", bufs=6) as pu1, \
             tc.tile_pool(name="pu0", bufs=6) as pu0, \
             tc.tile_pool(name="pjt", bufs=6) as pjt, \
             tc.tile_pool(name="ptmp", bufs=6) as ptmp, \
             tc.tile_pool(name="prt", bufs=6) as prt, \
             tc.tile_pool(name="ps", bufs=4, space="PSUM") as pspool:
            ce3 = cpool.tile([128, 384], F8E3, name="ce3_sb")
            djdr = cpool.tile([128, 2, 128], F8E5, name="djdr_sb")
            bandT = ce3[:, 0:128]
            bandTH = ce3[:, 128:256]
            negi4 = ce3[:, 256:384]
            consts_loaded = False

            for img in range(IMGS_PER_CORE):
                r0 = H * img
                for t in range(NTILES):
                    base = TS * t
                    M = min(TS, H - base)
                    K1 = min(M + 1, H - base)   # rows incl. bottom neighbor

                    u1t = pu1.tile([128, WP], F8E3, name="u1t")
                    nc.sync.dma_start(u1t[0:K1], u1d[r0 + base:r0 + base + K1, :])
                    u0t = pu0.tile([128, W], F8E3, name="u0t")
                    nc.sync.dma_start(u0t[0:M], u0d[r0 + base:r0 + base + M, :])
                    jt = pjt.tile([128, 4, 512], F8E4, name="jt")
                    nc.sync.dma_start(jt[0:M], jd[r0 + base:r0 + base + M])
                    if t == 0:
                        K, band = K1, bandT
                    else:
                        # top-neighbor u1 row rides at partition 127 (tiny
                        # SWDGE DMA: keep it off the serialized HWDGE device)
                        nc.gpsimd.dma_start(
                            u1t[127:128], u1d[r0 + base - 1:r0 + base, :])
                        K, band = 128, bandTH
                    if not consts_loaded:
                        # const loads ride the SWDGE ring (the serialized
                        # HWDGE device delays tile loads otherwise) after the
                        # first big loads so descriptor-gen feeds data at once
                        nc.gpsimd.dma_start(ce3[:], ce3_d.ap())
                        nc.gpsimd.dma_start(djdr[:], djdr_d.ap())
                        consts_loaded = True

                    # PSUM accumulates 4x everything linear except the
                    # horizontal neighbors: band@u1 - 4*u0 - 4cj*j2 + 4cj*j0.
                    ps = pspool.tile([128, W], F32, name="ps")
                    for h in range(2):
                        cs = slice(512 * h, 512 * h + 512)
                        nc.tensor.matmul(
                            ps[0:M, cs], band[0:K, 0:M],
                            u1t[0:K, 1 + 512 * h:513 + 512 * h],
                            start=True, stop=False,
                        )
                        nc.tensor.matmul(
                            ps[0:M, cs], negi4[0:M, 0:M], u0t[0:M, cs],
                            start=False, stop=False,
                        )
                        nc.tensor.matmul(
                            ps[0:M, cs], djdr[0:M, :, 0:M],
                            jt[0:M, 2 * h:2 * h + 2, :],
                            start=False, stop=True, perf_mode=DR,
                        )

                    # tmp = u1[., x-1] + u1[., x+1] (edge zero-pad via the
                    # host-padded columns; no scale needed at 4x)
                    tmp = ptmp.tile([128, W], BF16, name="tmp")
                    nc.vector.tensor_tensor(
                        tmp[0:M], u1t[0:M, 0:W], u1t[0:M, 2:WP], ALU.add)
                    # rt = psum, then rt += tmp
                    rt = prt.tile([128, W], BF16, name="rt")
                    nc.scalar.copy(rt[0:M], ps[0:M])
                    nc.vector.tensor_tensor(
                        rt[0:M], rt[0:M], tmp[0:M], ALU.add)

                    nc.gpsimd.dma_start(
                        outd[r0 + base:r0 + base + M, :], rt[0:M])

    nc.compile()
    return nc


_NC_CACHE = None


def _get_program():
    global _NC_CACHE
    if _NC_CACHE is None:
        _NC_CACHE = _build_program()
    return _NC_CACHE


def kernel(u1, u0, j2, j0):
    nc = _get_program()

    u1p = np.zeros((B, H, WP), dtype=NP_F8E3)
    u1p[:, :, 1:W + 1] = u1.reshape(B, H, W).astype(NP_F8E3)
    u0q = u0.reshape(B, H, W).astype(NP_F8E3)
    j2q = j2.reshape(B, H, W).astype(NP_F8E4)
    j0q = j0.reshape(B, H, W).astype(NP_F8E4)
    jq = np.empty((B, H, 4, 512), dtype=NP_F8E4)
    jq[:, :, 0, :] = j2q[:, :, 0:512]
    jq[:, :, 1, :] = j0q[:, :, 0:512]
    jq[:, :, 2, :] = j2q[:, :, 512:1024]
    jq[:, :, 3, :] = j0q[:, :, 512:1024]

    in_maps = []
    for c in range(NCORES):
        sl = slice(IMGS_PER_CORE * c, IMGS_PER_CORE * (c + 1))
        in_maps.append({
            "u1": np.ascontiguousarray(u1p[sl]).reshape(ROWS, WP),
            "u0": np.ascontiguousarray(u0q[sl]).reshape(ROWS, W),
            "jd": np.ascontiguousarray(jq[sl]).reshape(ROWS, 4, 512),
        })
    res = bass_utils.run_bass_kernel_spmd(nc, in_maps, core_ids=list(range(NCORES)))
    out = np.concatenate(
        [r["out"].reshape(IMGS_PER_CORE, 1, H, W) for r in res.results], axis=0
    )
    # undo the device-side 4x representation scale (exact in fp32)
    return (0.25 * out.astype(np.float32))


# revision 12
# speedup vs baseline: 1.1847x; 1.0963x over previous
"""Trainium2 Bass kernel for one FDM wave-equation step (5-point stencil CNN).

u2 = 2*u1 - u0 + 0.25*lap5(u1) - 0.0025*(j2 - j0)   on (16,1,1024,1024) f32.

Sharding: data-parallel over batch - 2 full images per NeuronCore. The result
tolerance (2e-2 L2) admits low-precision I/O, which is the main lever since
the problem is HBM-bandwidth bound (the TimelineSim cost model moves bytes at
360 GB/s through a serialized DMA-engine pool):

  u1  -> fp8 e3m4 (4 mantissa bits), zero-padded by one column each side
  u0 / j2 / j0 -> one packed uint8 tensor per row: u0 as e3m4 bytes in cols
         0:1024, then j2/j0 as e4m3 bytes interleaved [j2h0|j0h0|j2h1|j0h1]
         in 512-blocks. One DMA per tile; matmul APs bitcast the regions.
  out -> bf16, holding 4x the result; the host multiplies by 0.25 (exact)

The 4x output scale makes every device-side constant exact in fp8 with no
extra scaling pass: the stencil weights on raw u1 become {1, 4} (e3m4-exact),
u0's weight -4, and the horizontal neighbor sum u1[x-1]+u1[x+1] needs no
scale at all.

Per 126-row tile: all linear terms except the horizontal neighbors accumulate
in one PSUM group on the TensorEngine: the vertical stencil + center as a
banded-matrix matmul over the tile's u1 rows (the missing top-neighbor row is
stashed at partition 127 by a tiny Pool-ring DMA and fed to output row 0 by a
band entry at [127, 0]), u0 via a -4I matmul, and j2/j0 via a single fp8
DoubleRow matmul with -+4cj diagonal weights in e5m2 (2.3% off 0.01, which
shifts the 0.0025-weighted j-term by a negligible 5e-5 of the output). The
ACT engine drains PSUM to bf16 and the DVE adds the horizontal neighbor sum
(two tensor_tensor adds).

DMA-ring budget (every non-DMA resource must stay under the ~35us of DMA
transfer): loads ride the SP ring (2 HWDGE descriptor-gens per tile thanks to
the packing), the halo/const loads ride the Pool SWDGE ring, and stores are
issued two tiles late (so their triggers never park an in-order SEQ) and
alternate between the ACT HWDGE ring and the Pool SWDGE ring.

Measured end-to-end rel err vs the fp32 reference: ~1.4e-2 (limit 2e-2).
"""

import numpy as np
import ml_dtypes

import concourse.bacc as bacc
import concourse.mybir as mybir
import concourse.tile as tile
from concourse import bass_utils

F32 = mybir.dt.float32
BF16 = mybir.dt.bfloat16
U8 = mybir.dt.uint8
F8E3 = mybir.dt.float8e3
F8E4 = mybir.dt.float8e4
F8E5 = mybir.dt.float8e5
ALU = mybir.AluOpType
DR = mybir.MatmulPerfMode.DoubleRow
NP_BF16 = ml_dtypes.bfloat16
NP_F8E3 = ml_dtypes.float8_e3m4
NP_F8E4 = ml_dtypes.float8_e4m3
NP_F8E5 = ml_dtypes.float8_e5m2

H = W = 1024
B = 16
NCORES = 8
IMGS_PER_CORE = B // NCORES          # 2
ROWS = IMGS_PER_CORE * H             # 2048 rows per core
WP = W + 2                           # u1 padded width
TS = 126                             # output rows per tile
NTILES = (H + TS - 1) // TS          # 9
C_J = 0.0025                         # DT / (2*EPSILON)
STORE_DELAY = 2                      # tiles between rt ready and store issue


def _const_matrices():
    # bandT[k, m]: weight of u1 partition k (image row base+k) on 4x output
    # row m: {1, 4, 1} tridiagonal, all e3m4-exact. Top-edge zero-pad: row 0
    # has no k=-1 entry. Bottom-edge zero-pad falls out of slicing the
    # contraction down to the rows present.
    bandT = np.zeros((128, 128), dtype=NP_F8E3)
    for m in range(128):
        if m >= 1:
            bandT[m - 1, m] = NP_F8E3(1.0)
        bandT[m, m] = NP_F8E3(4.0)
        if m + 1 < 128:
            bandT[m + 1, m] = NP_F8E3(1.0)
    # bandTH: same, plus the top-neighbor row stashed at partition 127
    # feeding output row 0 (used for every tile but the first).
    bandTH = bandT.copy()
    bandTH[127, 0] = NP_F8E3(1.0)
    negi4 = (-4.0 * np.eye(128)).astype(NP_F8E3)
    ce3 = np.concatenate([bandT, bandTH, negi4], axis=1)   # one DMA
    # DoubleRow diag weights: k-tile 0 applies -4cj to j2, k-tile 1 +4cj to
    # j0 (on the 4x-scaled output).
    cj4 = np.float32(NP_F8E5(4 * C_J))
    djdr = np.zeros((128, 2, 128), dtype=NP_F8E5)
    for m in range(128):
        djdr[m, 0, m] = NP_F8E5(-cj4)
        djdr[m, 1, m] = NP_F8E5(cj4)
    return ce3, djdr


def _build_program():
    nc = bacc.Bacc(
        "TRN2",
        debug=False,
        enable_asserts=False,
        target_bir_lowering=False,
        num_devices=NCORES,
    )
    u1d = nc.dram_tensor("u1", [ROWS, WP], F8E3, kind="ExternalInput").ap()
    pkd = nc.dram_tensor("pk", [ROWS, 3 * W], U8, kind="ExternalInput").ap()
    outd = nc.dram_tensor("out", [ROWS, W], BF16, kind="ExternalOutput").ap()

    ce3_np, djdr_np = _const_matrices()
    ce3_d = nc.inline_tensor(ce3_np, name="ce3")
    djdr_d = nc.inline_tensor(djdr_np, name="djdr")

    with tile.TileContext(nc) as tc:
        with tc.tile_pool(name="consts", bufs=1) as cpool, \
             tc.tile_pool(name="pu1", bufs=6) as pu1, \
             tc.tile_pool(name="ppk", bufs=6) as ppk, \
             tc.tile_pool(name="ptmp", bufs=6) as ptmp, \
             tc.tile_pool(name="prt", bufs=6 + STORE_DELAY) as prt, \
             tc.tile_pool(name="ps", bufs=4, space="PSUM") as pspool:
            ce3 = cpool.tile([128, 384], F8E3, name="ce3_sb")
            djdr = cpool.tile([128, 2, 128], F8E5, name="djdr_sb")
            bandT = ce3[:, 0:128]
            bandTH = ce3[:, 128:256]
            negi4 = ce3[:, 256:384]
            consts_loaded = False

            pending = []   # (tile_idx, rt slice, dram row range)

            def flush(keep):
                while len(pending) > keep:
                    i, rt_, rows_ = pending.pop(0)
                    ring = nc.scalar if i % 2 == 0 else nc.gpsimd
                    ring.dma_start(outd[rows_[0]:rows_[1], :], rt_)

            ti = 0
            for img in range(IMGS_PER_CORE):
                r0 = H * img
                for t in range(NTILES):
                    base = TS * t
                    M = min(TS, H - base)
                    K1 = min(M + 1, H - base)   # rows incl. bottom neighbor

                    u1t = pu1.tile([128, WP], F8E3, name="u1t")
                    nc.sync.dma_start(u1t[0:K1], u1d[r0 + base:r0 + base + K1, :])
                    pkt = ppk.tile([128, 3 * W], U8, name="pkt")
                    nc.sync.dma_start(pkt[0:M], pkd[r0 + base:r0 + base + M, :])
                    if t == 0:
                        K, band = K1, bandT
                    else:
                        # top-neighbor u1 row rides at partition 127 (tiny
                        # SWDGE DMA: keep it off the serialized HWDGE device)
                        nc.gpsimd.dma_start(
                            u1t[127:128], u1d[r0 + base - 1:r0 + base, :])
                        K, band = 128, bandTH
                    if not consts_loaded:
                        # const loads ride the SWDGE ring (the serialized
                        # HWDGE device delays tile loads otherwise) after the
                        # first big loads so descriptor-gen feeds data at once
                        nc.gpsimd.dma_start(ce3[:], ce3_d.ap())
                        nc.gpsimd.dma_start(djdr[:], djdr_d.ap())
                        consts_loaded = True

                    # PSUM accumulates 4x everything linear except the
                    # horizontal neighbors: band@u1 - 4*u0 - 4cj*j2 + 4cj*j0.
                    ps = pspool.tile([128, W], F32, name="ps")
                    for h in range(2):
                        cs = slice(512 * h, 512 * h + 512)
                        u0v = pkt[0:M, 512 * h:512 * h + 512].bitcast(F8E3)
                        jv = (pkt[0:M, 1024 + 1024 * h:2048 + 1024 * h]
                              .bitcast(F8E4)
                              .rearrange("p (a c) -> p a c", a=2, c=512))
                        nc.tensor.matmul(
                            ps[0:M, cs], band[0:K, 0:M],
                            u1t[0:K, 1 + 512 * h:513 + 512 * h],
                            start=True, stop=False,
                        )
                        nc.tensor.matmul(
                            ps[0:M, cs], negi4[0:M, 0:M], u0v,
                            start=False, stop=False,
                        )
                        nc.tensor.matmul(
                            ps[0:M, cs], djdr[0:M, :, 0:M], jv,
                            start=False, stop=True, perf_mode=DR,
                        )

                    # tmp = u1[., x-1] + u1[., x+1] (edge zero-pad via the
                    # host-padded columns; no scale needed at 4x)
                    tmp = ptmp.tile([128, W], BF16, name="tmp")
                    nc.vector.tensor_tensor(
                        tmp[0:M], u1t[0:M, 0:W], u1t[0:M, 2:WP], ALU.add)
                    # rt = psum, then rt += tmp
                    rt = prt.tile([128, W], BF16, name="rt")
                    nc.scalar.copy(rt[0:M], ps[0:M])
                    nc.vector.tensor_tensor(
                        rt[0:M], rt[0:M], tmp[0:M], ALU.add)

                    pending.append((ti, rt[0:M], (r0 + base, r0 + base + M)))
                    flush(STORE_DELAY)
                    ti += 1
            flush(0)

    nc.compile()
    return nc


_NC_CACHE = None


def _get_program():
    global _NC_CACHE
    if _NC_CACHE is None:
        _NC_CACHE = _build_program()
    return _NC_CACHE


def kernel(u1, u0, j2, j0):
    nc = _get_program()

    u1p = np.zeros((B, H, WP), dtype=NP_F8E3)
    u1p[:, :, 1:W + 1] = u1.reshape(B, H, W).astype(NP_F8E3)
    j2q = j2.reshape(B, H, W).astype(NP_F8E4)
    j0q = j0.reshape(B, H, W).astype(NP_F8E4)
    pk = np.empty((B, H, 3 * W), dtype=np.uint8)
    pk[:, :, 0:W] = u0.reshape(B, H, W).astype(NP_F8E3).view(np.uint8)
    pk[:, :, W + 0 * 512:W + 1 * 512] = j2q[:, :, 0:512].view(np.uint8)
    pk[:, :, W + 1 * 512:W + 2 * 512] = j0q[:, :, 0:512].view(np.uint8)
    pk[:, :, W + 2 * 512:W + 3 * 512] = j2q[:, :, 512:1024].view(np.uint8)
    pk[:, :, W + 3 * 512:W + 4 * 512] = j0q[:, :, 512:1024].view(np.uint8)

    in_maps = []
    for c in range(NCORES):
        sl = slice(IMGS_PER_CORE * c, IMGS_PER_CORE * (c + 1))
        in_maps.append({
            "u1": np.ascontiguousarray(u1p[sl]).reshape(ROWS, WP),
            "pk": np.ascontiguousarray(pk[sl]).reshape(ROWS, 3 * W),
        })
    res = bass_utils.run_bass_kernel_spmd(nc, in_maps, core_ids=list(range(NCORES)))
    out = np.concatenate(
        [r["out"].reshape(IMGS_PER_CORE, 1, H, W) for r in res.results], axis=0
    )
    # undo the device-side 4x representation scale (exact in fp32)
    return (0.25 * out.astype(np.float32))
